# revision 1
# baseline (speedup 1.0000x reference)
"""Trainium2 Bass kernel for nn_AccumulateLoss — v2 (element-major layout).

Math (CONTINLEN=5 -> 10 pairs, 10 triples (i,k,j), batch B=262144):
  fuse_rota  = R[ik] @ R[kj]            (batched 3x3 matmul)
  fuse_trans = R[ik] @ t[ik] + t[kj]
  loss = 50 * sum((fuse_rota - R[ij])^2) + sum((fuse_trans - t[ij])^2)

Key change vs v1: the host pre-transposes each core's shard to
element-major layout  R[q, p, (i,j), f]  (batch f innermost: 9 matrix
slots per pair, each a run of F contiguous bf16 values per partition).
Every DVE tensor_tensor then has ALL operands with innermost stride 1
on 2-byte data -> the cost model's 2x perf mode applies to the
broadcast matmul products too (in v1's batch-major layout the
broadcast operand had innermost stride 0 -> 1x).

Per-pair R tiles give pair-granular DMA->compute dependency, so
compute starts after the first few pair DMAs and the remaining DMA
(~24us total) hides under the DVE work. transs stays one tile (small,
DMA'd early, used only by the trailing trans section).

Squares+batch-reduction ride free on ScalarE via activation(Square,
accum_out). Optional GPSIMD (Pool) offload (tunable; DVE+Pool share
SBUF ports on real HW so the win must be measured, not assumed).
"""
import numpy as np

# ---- problem constants (hardcoded; kernel must be self-contained) ----
N_CORES = 8
CONTINLEN = 5
NPAIR = 10
B_FULL = 262144
B_CORE = B_FULL // N_CORES       # 32768
P = 128                          # SBUF partitions
F_TOT = B_CORE // P              # 256 batch slots per partition
BETA = 50.0

# ---- tunables ----
NCHUNK = 1                       # batch chunks (1 = single shot)
F = F_TOT // NCHUNK
POOL_TRANS = False               # all trans-part ops on GPSIMD (Pool)
POOL_TRANS_D = False             # only the d=t2-t12 / diff ops on Pool
POOL_K2T = 0                     # triples whose k=2 product goes on Pool


def _pair_id():
    pid = {}
    p = 0
    for a in range(CONTINLEN):
        for b in range(a + 1, CONTINLEN):
            pid[(a, b)] = p
            p += 1
    return pid


_PID = _pair_id()

# groups of triples (i,k,j) sharing (i,k); j in [k+1, CONTINLEN)
GROUPS = []
for _i in range(CONTINLEN):
    for _k in range(_i + 1, CONTINLEN - 1):
        GROUPS.append((_PID[(_i, _k)], _PID[(_k, _k + 1)],
                       _PID[(_i, _k + 1)], CONTINLEN - 1 - _k))
NGRP = len(GROUPS)               # 6
TRIPLES = []
for _g, (_q1, _q2, _q12, _G) in enumerate(GROUPS):
    for _j in range(_G):
        TRIPLES.append((_q1, _q2 + _j, _q12 + _j))
T = len(TRIPLES)                 # 10
NCOL_C = T + NGRP                # 16 loss cols per chunk
NCOL = NCOL_C * NCHUNK

_NC_CACHE = {}


def _build_nc(repeat=1):
    import concourse.tile as tile
    from concourse import bacc, mybir

    nc = bacc.Bacc("TRN2", target_bir_lowering=False, debug=False,
                   num_devices=N_CORES)
    bf16 = mybir.dt.bfloat16
    f32 = mybir.dt.float32
    r_ext = nc.declare_dram_parameter(
        "rotas", [NPAIR, P, 9 * F_TOT], bf16, isOutput=False)
    t_ext = nc.declare_dram_parameter(
        "transs", [NPAIR, P, 3 * F_TOT], bf16, isOutput=False)
    out_ext = nc.declare_dram_parameter(
        "out", [P, NCOL], f32, isOutput=True)

    mult = mybir.AluOpType.mult
    add = mybir.AluOpType.add
    sub = mybir.AluOpType.subtract
    SQ = mybir.ActivationFunctionType.Square

    r_view = r_ext.ap()              # [q, p, 9*F_TOT]
    t_view = t_ext.ap()

    # R pair DMA order: by first use across the triples
    r_first_use = []
    for tri in TRIPLES:
        for q in tri:
            if q not in r_first_use:
                r_first_use.append(q)

    with tile.TileContext(nc) as tc:
        with tc.tile_pool(name="data", bufs=2) as data_pool, \
             tc.tile_pool(name="work", bufs=4) as work_pool, \
             tc.tile_pool(name="acc", bufs=1) as acc_pool:
            loss = acc_pool.tile([P, NCOL], f32)
            dbufs = 1 if NCHUNK == 1 else 2

            def emit_chunk(c):
                Rt = {q: data_pool.tile([P, 9 * F], bf16, tag=f"R{q}",
                                        name=f"R{q}", bufs=dbufs)
                      for q in range(NPAIR)}
                Tbuf = data_pool.tile([P, NPAIR * 3 * F], bf16, tag="Tbuf",
                                      bufs=dbufs)

                def dma_r(q):
                    if NCHUNK == 1:
                        nc.sync.dma_start(Rt[q][:], r_view[q])
                    else:
                        src = r_view[q].rearrange(
                            "p (e f) -> p e f", e=9)[:, :, c * F:(c + 1) * F]
                        dst = Rt[q][:].rearrange("p (e f) -> p e f", e=9)
                        nc.sync.dma_start(dst, src)

                def dma_t(q):
                    if NCHUNK == 1:
                        nc.sync.dma_start(
                            Tbuf[:, q * 3 * F:(q + 1) * 3 * F], t_view[q])
                    else:
                        src = t_view[q].rearrange(
                            "p (e f) -> p e f", e=3)[:, :, c * F:(c + 1) * F]
                        dst = Tbuf[:, q * 3 * F:(q + 1) * 3 * F].rearrange(
                            "p (e f) -> p e f", e=3)
                        nc.sync.dma_start(dst, src)

                for q in r_first_use[:4]:
                    dma_r(q)
                for q in range(NPAIR):
                    dma_t(q)
                for q in r_first_use[4:]:
                    dma_r(q)

                def R5(q):                      # [p, i, j, f]
                    return Rt[q][:].rearrange("p (i j f) -> p i j f",
                                              i=3, j=3, f=F)

                T4 = Tbuf[:].rearrange("p (q i f) -> p q i f",
                                       q=NPAIR, i=3, f=F)

                # ---------------- rota: per triple ----------------
                for t, (i1, i2, i12) in enumerate(TRIPLES):
                    tp0 = work_pool.tile([P, 9 * F], bf16, tag="tp0")
                    tp1 = work_pool.tile([P, 9 * F], bf16, tag="tp1")
                    tp2 = work_pool.tile([P, 9 * F], bf16, tag="tp2")
                    for k, pt in ((0, tp0), (1, tp1), (2, tp2)):
                        # p_k[i,j,f] = R1[i,k,f] * R2[k,j,f]
                        in0 = R5(i1)[:, :, k, :].unsqueeze(2) \
                            .broadcast_to([P, 3, 3, F])
                        in1 = R5(i2)[:, k, :, :].unsqueeze(1) \
                            .broadcast_to([P, 3, 3, F])
                        out = pt[:].rearrange("p (i j f) -> p i j f",
                                              i=3, j=3, f=F)
                        eng = nc.gpsimd if (k == 2 and t < POOL_K2T) \
                            else nc.vector
                        eng.tensor_tensor(out, in0, in1, mult)
                    tpA = work_pool.tile([P, 9 * F], bf16, tag="tpA")
                    nc.vector.tensor_tensor(tpA[:], tp0[:], tp1[:], add)
                    nc.vector.tensor_tensor(tp0[:], tpA[:], tp2[:], add)
                    nc.vector.tensor_tensor(tpA[:], tp0[:], Rt[i12][:], sub)
                    col = c * NCOL_C + t
                    nc.scalar.activation(tpA[:], tpA[:], SQ,
                                         accum_out=loss[:, col:col + 1])

                # ---------------- trans: per group ----------------
                teng = nc.gpsimd if POOL_TRANS else nc.vector
                for g, (q1, q2_0, q12_0, G) in enumerate(GROUPS):
                    # v = R1 @ t1  [p, i, f]
                    v0 = work_pool.tile([P, 3 * F], bf16, tag="v0")
                    v1 = work_pool.tile([P, 3 * F], bf16, tag="v1")
                    v2 = work_pool.tile([P, 3 * F], bf16, tag="v2")
                    for j, vt in ((0, v0), (1, v1), (2, v2)):
                        in0 = R5(q1)[:, :, j, :]                  # [p, i, f]
                        in1 = T4[:, q1, j, :].unsqueeze(1) \
                            .broadcast_to([P, 3, F])
                        out = vt[:].rearrange("p (i f) -> p i f", i=3, f=F)
                        teng.tensor_tensor(out, in0, in1, mult)
                    teng.tensor_tensor(v0[:], v0[:], v1[:], add)
                    teng.tensor_tensor(v0[:], v0[:], v2[:], add)
                    # d = t2 - t12 for the whole group, then diff = v + d
                    dg = work_pool.tile([P, G * 3 * F], bf16, tag="dg")
                    d4 = dg[:].rearrange("p (g i f) -> p g i f",
                                         g=G, i=3, f=F)
                    t2g = T4[:, q2_0:q2_0 + G]
                    t12g = T4[:, q12_0:q12_0 + G]
                    teng.tensor_tensor(d4, t2g, t12g, sub)
                    vbc = v0[:].rearrange("p (i f) -> p i f", i=3, f=F) \
                        .unsqueeze(1).broadcast_to([P, G, 3, F])
                    teng.tensor_tensor(d4, vbc, d4, add)
                    col = c * NCOL_C + T + g
                    nc.scalar.activation(dg[:], dg[:], SQ,
                                         accum_out=loss[:, col:col + 1])

            if repeat > 1:
                with tc.For_i(0, repeat, 1):
                    for c in range(NCHUNK):
                        emit_chunk(c)
            else:
                for c in range(NCHUNK):
                    emit_chunk(c)

            nc.sync.dma_start(out_ext.ap(), loss[:])

    nc.compile()
    return nc


def _get_nc(repeat=1):
    key = ("nc", repeat)
    if key not in _NC_CACHE:
        _NC_CACHE[key] = _build_nc(repeat)
    return _NC_CACHE[key]


def make_in_maps(rotas, transs):
    """Slice per core and host-transpose to element-major bf16.

    rotas [Q, B, 3, 3] -> per core [Q, P, 9, F_TOT] (batch innermost)
    transs [Q, B, 3]   -> per core [Q, P, 3, F_TOT]
    """
    import ml_dtypes
    rotas = np.asarray(rotas)
    transs = np.asarray(transs)
    in_maps = []
    for c in range(N_CORES):
        sl = slice(c * B_CORE, (c + 1) * B_CORE)
        r = rotas[:, sl].reshape(NPAIR, P, F_TOT, 9).transpose(0, 1, 3, 2)
        t = transs[:, sl].reshape(NPAIR, P, F_TOT, 3).transpose(0, 1, 3, 2)
        in_maps.append({
            "rotas": np.ascontiguousarray(r).astype(ml_dtypes.bfloat16)
                       .reshape(NPAIR, P, 9 * F_TOT),
            "transs": np.ascontiguousarray(t).astype(ml_dtypes.bfloat16)
                        .reshape(NPAIR, P, 3 * F_TOT),
        })
    return in_maps


def run_on_cores(rotas, transs):
    from concourse.bass_utils import run_bass_kernel_spmd

    nc = _get_nc()
    in_maps = make_in_maps(rotas, transs)
    res = run_bass_kernel_spmd(nc, in_maps, core_ids=list(range(N_CORES)))
    cols = np.stack([np.asarray(res.results[i]["out"])
                     for i in range(N_CORES)])
    return cols, res


def _reduce_cols(cols):
    """cols: [n_cores, P, NCOL] -> scalar loss (float64 host reduction)."""
    v = cols.astype(np.float64).reshape(-1, NCHUNK, NCOL_C)
    rota = v[:, :, :T].sum()
    trans = v[:, :, T:].sum()
    return rota * BETA + trans


def kernel(rotas, transs):
    rotas = np.asarray(rotas)
    transs = np.asarray(transs)
    cols, _ = run_on_cores(rotas, transs)
    return np.array([_reduce_cols(cols)], dtype=np.float32)

